# revision 8
# baseline (speedup 1.0000x reference)
"""Causal gated linear attention on 8 Trainium2 NeuronCores.

Sharding: head-parallel (16 heads / 8 cores = 2 heads per core, per the
sharding hint).  Each core computes q/k/v/gate columns for its 2 heads
(contiguous 128-dim slices), runs chunked causal linear attention for those
heads, and produces a partial projection y_c = O_c @ proj_w[:, cols].T.
The host sums the 8 partials (+ proj_b) — the "all-reduce at the final
proj" from the hint, done as the unshard step.

The gate normalizer mean_t = mean_j sigmoid(x @ gate_w.T + gate_b)_j needs
all 1024 gate columns, so it is exchanged between cores: launch 1 computes
each core's 128-column partial column-sums (plus gate_sig itself), the host
adds the 8 tiny [2048] vectors, and launch 2 does everything else.  (An
on-device AllReduce was measured at ~70us latency on this ring — the extra
NEFF tail barrier + host add is cheaper.)

LayerNorm is folded into the matmuls: launch 2 computes per-row mu/var with
bn_stats on x, and the qkv matmuls run on raw x^T with a K=2 correction
matmul ([colsum(g*W); b-dot-W] x [-mu; 1/rstd]) plus a per-row rstd scale
folded into the gate/epilogue multiplies.
"""

import contextlib
import os
import sys
import types

import numpy as np
import ml_dtypes

import concourse.bass as bass
import concourse.mybir as mybir
import concourse.tile as tile
from concourse import bacc
from concourse.bass_utils import run_bass_kernel_spmd

BF16 = mybir.dt.bfloat16
F32 = mybir.dt.float32
AF = mybir.ActivationFunctionType
ALU = mybir.AluOpType

D = 1024
T = 2048
H = 16
DH = 64
NCORES = 8
OPC = 128          # output columns per core (2 heads x 64)
C = 128            # attention chunk
NCH = T // C       # 16 chunks
NKT = D // 128     # 8 k-tiles of x^T
NTC = T // 512     # 4 moving chunks of 512
EPS_LN = 1e-5
EPS_GATE = 1e-5
EPS_DEN = 1e-5


def _install_profile_hook():
    """NTFF profile hook (only used by test.py via trace=True; harmless)."""
    if "antenv.axon_hooks" in sys.modules:
        return
    try:
        from trn_agent_boot.trn_boot import _ntff_profile_via_ctypes

        hook = _ntff_profile_via_ctypes("/opt/axon/libaxon_pjrt.so")
        mod = types.ModuleType("antenv.axon_hooks")
        mod.get_axon_ntff_profile_hook = lambda: hook
        sys.modules["antenv.axon_hooks"] = mod
    except Exception:
        pass


# ----------------------------------------------------------------------------
# launch 1: gate matmul + sigmoid + per-core column-sum partials
# ----------------------------------------------------------------------------

def build_launch1():
    nc = bacc.Bacc("TRN2")
    xT = nc.dram_tensor("xT", [D, T], BF16, kind="ExternalInput")
    wg = nc.dram_tensor("wg", [D, OPC], BF16, kind="ExternalInput")
    gb = nc.dram_tensor("gb", [OPC, 1], F32, kind="ExternalInput")
    gsig_d = nc.dram_tensor("gsig", [OPC, T], BF16, kind="ExternalOutput")
    gcs_d = nc.dram_tensor("gcs", [1, T], F32, kind="ExternalOutput")

    with tile.TileContext(nc) as tc:
        with (
            tc.tile_pool(name="sb", bufs=1) as sb,
            tc.tile_pool(name="ps", bufs=4, space="PSUM") as ps,
            tc.tile_pool(name="ps1", bufs=4, space="PSUM") as ps1,
        ):
            xts = []
            for k in range(NKT):
                t = sb.tile([128, T], BF16, tag=f"xt{k}")
                nc.sync.dma_start(out=t, in_=xT[128 * k : 128 * k + 128, :])
                xts.append(t)
            wgs = []
            for k in range(NKT):
                t = sb.tile([128, OPC], BF16, tag=f"wg{k}")
                nc.sync.dma_start(out=t, in_=wg[128 * k : 128 * k + 128, :])
                wgs.append(t)
            gbs = sb.tile([OPC, 1], F32, tag="gb")
            nc.sync.dma_start(out=gbs, in_=gb[:, :])
            ones_col = sb.tile([128, 1], BF16, tag="ones")
            nc.vector.memset(ones_col, 1.0)

            gsig = sb.tile([OPC, T], BF16, tag="gsig")
            gcs_row = sb.tile([1, T], F32, tag="gcs")
            for j in range(NTC):
                pg = ps.tile([128, 512], F32, tag="pg")
                for k in range(NKT):
                    nc.tensor.matmul(
                        pg[:, :],
                        wgs[k][:, :],
                        xts[k][:, 512 * j : 512 * j + 512],
                        start=(k == 0),
                        stop=(k == NKT - 1),
                    )
                nc.scalar.activation(
                    out=gsig[:, 512 * j : 512 * j + 512],
                    in_=pg[:, :],
                    func=AF.Sigmoid,
                    bias=gbs[:, :],
                    scale=1.0,
                )
                pc = ps1.tile([1, 512], F32, tag="pc")
                nc.tensor.matmul(
                    pc[:, :],
                    ones_col[:, :],
                    gsig[:, 512 * j : 512 * j + 512],
                    start=True,
                    stop=True,
                )
                nc.vector.tensor_copy(gcs_row[:, 512 * j : 512 * j + 512], pc[:, :])
            nc.sync.dma_start(out=gsig_d[:, :], in_=gsig[:, :])
            nc.sync.dma_start(out=gcs_d[:, :], in_=gcs_row[:, :])
    nc.finalize()
    return nc


# ----------------------------------------------------------------------------
# launch 2: everything else
# ----------------------------------------------------------------------------

def build_launch2():
    nc = bacc.Bacc("TRN2")
    xT = nc.dram_tensor("xT", [D, T], BF16, kind="ExternalInput")
    xn = nc.dram_tensor("xn", [T, D], BF16, kind="ExternalInput")
    gsig_d = nc.dram_tensor("gsig", [OPC, T], BF16, kind="ExternalInput")
    grec_d = nc.dram_tensor("grec", [128, NCH], F32, kind="ExternalInput")
    wq = nc.dram_tensor("wq", [D, OPC], BF16, kind="ExternalInput")
    wk = nc.dram_tensor("wk", [D, OPC], BF16, kind="ExternalInput")
    wv = nc.dram_tensor("wv", [D, OPC], BF16, kind="ExternalInput")
    # K=2 LN-correction operands: rows [colsum(g*W) ; b-dot-W + bias]
    cgq = nc.dram_tensor("cgq", [2, OPC], BF16, kind="ExternalInput")
    cgk = nc.dram_tensor("cgk", [2, OPC], BF16, kind="ExternalInput")
    cgv = nc.dram_tensor("cgv", [2, OPC], BF16, kind="ExternalInput")
    pw = nc.dram_tensor("pw", [OPC, D], BF16, kind="ExternalInput")
    mask_d = nc.dram_tensor("mask", [C, C], F32, kind="ExternalInput")
    ident_d = nc.dram_tensor("ident", [128, 128], BF16, kind="ExternalInput")
    y_d = nc.dram_tensor("y", [T, D], F32, kind="ExternalOutput")

    # DRAM bounce buffers for [128, 16] <-> [1, 2048] reshapes
    b_negmu = nc.dram_tensor("b_negmu", [T], BF16)
    b_irstd = nc.dram_tensor("b_irstd", [T], BF16)
    b_gnb = nc.dram_tensor("b_gnb", [T], BF16)

    with tile.TileContext(nc) as tc:
        with contextlib.ExitStack() as ctx:
            sb = ctx.enter_context(tc.tile_pool(name="sb", bufs=1))
            xpool = ctx.enter_context(tc.tile_pool(name="xp", bufs=3))
            work = ctx.enter_context(tc.tile_pool(name="work", bufs=3))
            ypool = ctx.enter_context(tc.tile_pool(name="yp", bufs=3))

            # ---- static loads -------------------------------------------------
            xts = []
            for k in range(NKT):
                t = sb.tile([128, T], BF16, tag=f"xt{k}")
                nc.sync.dma_start(out=t, in_=xT[128 * k : 128 * k + 128, :])
                xts.append(t)
            wqs, wks, wvs = [], [], []
            for k in range(NKT):
                for nm, dr, lst in (("q", wq, wqs), ("k", wk, wks), ("v", wv, wvs)):
                    t = sb.tile([128, OPC], BF16, tag=f"w{nm}{k}")
                    nc.sync.dma_start(out=t, in_=dr[128 * k : 128 * k + 128, :])
                    lst.append(t)
            cgqs = sb.tile([2, OPC], BF16, tag="cgq")
            cgks = sb.tile([2, OPC], BF16, tag="cgk")
            cgvs = sb.tile([2, OPC], BF16, tag="cgv")
            nc.sync.dma_start(out=cgqs, in_=cgq[:, :])
            nc.sync.dma_start(out=cgks, in_=cgk[:, :])
            nc.sync.dma_start(out=cgvs, in_=cgv[:, :])
            gsig = sb.tile([OPC, T], BF16, tag="gsig")
            nc.sync.dma_start(out=gsig, in_=gsig_d[:, :])
            grec = sb.tile([128, NCH], F32, tag="grec")
            nc.sync.dma_start(out=grec, in_=grec_d[:, :])
            pws = sb.tile([OPC, D], BF16, tag="pw")
            nc.sync.dma_start(out=pws, in_=pw[:, :])
            mask = sb.tile([C, C], F32, tag="mask")
            nc.sync.dma_start(out=mask, in_=mask_d[:, :])
            ident = sb.tile([128, 128], BF16, tag="ident")
            nc.sync.dma_start(out=ident, in_=ident_d[:, :])
            ones_col = sb.tile([128, 1], BF16, tag="onesc")
            nc.vector.memset(ones_col, 1.0)
            ones_row64 = sb.tile([1, 64], BF16, tag="ones64")
            nc.vector.memset(ones_row64, 1.0)
            ones_row128 = sb.tile([1, 128], BF16, tag="ones128")
            nc.vector.memset(ones_row128, 1.0)
            eps_ln = sb.tile([128, 1], F32, tag="epsln")
            nc.vector.memset(eps_ln, EPS_LN)
            eps_den = sb.tile([1, 1], F32, tag="epsden")
            nc.vector.memset(eps_den, EPS_DEN)

            # ---- layernorm stats over natural-layout x ------------------------
            mu_nat = sb.tile([128, NCH], F32, tag="mu")
            var_nat = sb.tile([128, NCH], F32, tag="var")
            for i in range(NCH):
                xt = xpool.tile([128, D], BF16, tag="xrow")
                nc.sync.dma_start(out=xt, in_=xn[128 * i : 128 * i + 128, :])
                st = work.tile([128, 2, 6], F32, tag="bnst")
                nc.vector.bn_stats(out=st[:, 0, :], in_=xt[:, 0:512])
                nc.vector.bn_stats(out=st[:, 1, :], in_=xt[:, 512:1024])
                mv = work.tile([128, 2], F32, tag="bnmv")
                nc.vector.bn_aggr(out=mv[:, :], in_=st[:, :, :])
                nc.vector.tensor_copy(mu_nat[:, i : i + 1], mv[:, 0:1])
                nc.vector.tensor_copy(var_nat[:, i : i + 1], mv[:, 1:2])
            irstd_f32 = sb.tile([128, NCH], F32, tag="irstdf")
            nc.scalar.activation(
                out=irstd_f32, in_=var_nat, func=AF.Sqrt, bias=eps_ln[:, :], scale=1.0
            )
            rstd_nat = sb.tile([128, NCH], F32, tag="rstd")
            nc.vector.reciprocal(rstd_nat, irstd_f32)
            negmu_nat = sb.tile([128, NCH], BF16, tag="negmu")
            nc.vector.tensor_scalar_mul(negmu_nat, mu_nat, -1.0)
            irstd_nat = sb.tile([128, NCH], BF16, tag="irstd")
            nc.vector.tensor_copy(irstd_nat, irstd_f32)
            gnb_nat = sb.tile([128, NCH], BF16, tag="gnbn")
            nc.vector.tensor_mul(gnb_nat, rstd_nat, grec)

            # nat [128, 16] -> row [1, 2048] via DRAM bounce (t = i*128 + p)
            ex = sb.tile([2, T], BF16, tag="ex")  # rows: [-mu ; 1/rstd]
            gnb_row = sb.tile([1, T], BF16, tag="gnbr")
            for nat, bounce, dst in (
                (negmu_nat, b_negmu, ex[0:1, :]),
                (irstd_nat, b_irstd, ex[1:2, :]),
                (gnb_nat, b_gnb, gnb_row[:, :]),
            ):
                br = bounce.ap().rearrange("(i p) -> p i", p=128)
                nc.sync.dma_start(out=br, in_=nat[:, :])
                nc.sync.dma_start(out=dst, in_=bounce.ap().rearrange("(o t) -> o t", o=1))

            qT = sb.tile([OPC, T], BF16, tag="qT")
            kT = sb.tile([OPC, T], BF16, tag="kT")
            v_nat = sb.tile([128, NCH, OPC], BF16, tag="vnat")
            k_nat = sb.tile([128, NCH, OPC], BF16, tag="knat")
            gate_eff = sb.tile([OPC, T], F32, tag="geff")

            # ---- phase 1: q/k/v raw matmuls + epilogues -----------------------
            with (
                tc.tile_pool(name="qk", bufs=3, space="PSUM") as qkp,
                tc.tile_pool(name="vp", bufs=2, space="PSUM") as vp,
                tc.tile_pool(name="bc", bufs=2, space="PSUM") as bcp,
            ):
                for j in range(NTC):
                    sl = slice(512 * j, 512 * j + 512)
                    # gate_eff = gate_sig * bcast(grec * rstd)
                    gb_ps = bcp.tile([128, 512], F32, tag="gbc")
                    nc.tensor.matmul(
                        gb_ps[:, :],
                        ones_row128[:, :],
                        gnb_row[:, sl],
                        start=True,
                        stop=True,
                    )
                    gsl = work.tile([128, 512], F32, tag="gsl")
                    nc.scalar.activation(
                        out=gsl, in_=gsig[:, sl], func=AF.Copy, bias=0.0, scale=1.0
                    )
                    nc.vector.tensor_mul(gate_eff[:, sl], gsl, gb_ps)

                    for nm, ws, cgs, outT in (("q", wqs, cgqs, qT), ("k", wks, cgks, kT)):
                        pq = qkp.tile([128, 512], F32, tag="qk")
                        for k in range(NKT):
                            nc.tensor.matmul(
                                pq[:, :], ws[k][:, :], xts[k][:, sl],
                                start=(k == 0), stop=False,
                            )
                        nc.tensor.matmul(
                            pq[:, :], cgs[:, :], ex[:, sl], start=False, stop=True,
                        )
                        q1 = work.tile([128, 512], F32, tag="q1")
                        nc.vector.tensor_mul(q1, pq, gate_eff[:, sl])
                        e1 = work.tile([128, 512], F32, tag="e1")
                        nc.scalar.activation(out=e1, in_=q1, func=AF.Exp, bias=0.0, scale=1.0)
                        e2 = work.tile([128, 512], F32, tag="e2")
                        nc.vector.tensor_scalar_min(e2, e1, 1.0)
                        nc.vector.scalar_tensor_tensor(
                            out=outT[:, sl], in0=q1, scalar=0.0, in1=e2,
                            op0=ALU.max, op1=ALU.add,
                        )
                # v natural: [t, o] tiles
                for i in range(NCH):
                    pv = vp.tile([128, OPC], F32, tag="v")
                    tsl = slice(128 * i, 128 * i + 128)
                    for k in range(NKT):
                        nc.tensor.matmul(
                            pv[:, :], xts[k][:, tsl], wvs[k][:, :],
                            start=(k == 0), stop=False,
                        )
                    nc.tensor.matmul(
                        pv[:, :], ex[:, tsl], cgvs[:, :], start=False, stop=True,
                    )
                    nc.vector.tensor_scalar_mul(
                        v_nat[:, i, :], pv, rstd_nat[:, i : i + 1]
                    )

            # ---- phase 2a: transpose kT chunks -> k natural -------------------
            with tc.tile_pool(name="tr", bufs=2, space="PSUM") as trp:
                for i in range(NCH):
                    pt = trp.tile([128, 128], BF16, tag="tr")
                    nc.tensor.transpose(
                        pt[:, :], kT[:, 128 * i : 128 * i + 128], ident[:, :]
                    )
                    nc.vector.tensor_copy(k_nat[:, i, :], pt[:, :])

            # z prefix sums (exclusive, per k-dim row): [128, NCH]
            z_all = sb.tile([OPC, NCH], F32, tag="zall")
            nc.vector.tensor_reduce(
                out=z_all,
                in_=kT[:, :].rearrange("p (c t) -> p c t", c=NCH),
                axis=mybir.AxisListType.X,
                op=ALU.add,
            )
            z_exc = sb.tile([OPC, NCH], BF16, tag="zexc")
            z_run = work.tile([OPC, 1], F32, tag="zrun")
            nc.vector.memset(z_exc[:, 0:1], 0.0)
            nc.vector.tensor_copy(z_run, z_all[:, 0:1])
            for i in range(1, NCH):
                nc.vector.tensor_copy(z_exc[:, i : i + 1], z_run)
                if i < NCH - 1:
                    nc.vector.tensor_add(z_run, z_run, z_all[:, i : i + 1])

            # ---- phase 2b: attention + projection -----------------------------
            oT = sb.tile([OPC, T], BF16, tag="oT")
            with (
                tc.tile_pool(name="st", bufs=2, space="PSUM") as stp,
                tc.tile_pool(name="ot", bufs=1, space="PSUM") as otp,
                tc.tile_pool(name="den", bufs=1, space="PSUM") as denp,
                tc.tile_pool(name="sta", bufs=1, space="PSUM") as stap,
                tc.tile_pool(name="bc2", bufs=1, space="PSUM") as bcp2,
                tc.tile_pool(name="pj", bufs=2, space="PSUM") as pjp,
            ):
                state_ps = stap.tile([128, 64], F32, tag="state")
                state_sb = None
                for g in range(NTC):  # groups of 4 chunks
                    ot_ps = otp.tile([128, 512], F32, tag="ot")
                    den_ps = denp.tile([128, 512], F32, tag="den")
                    for cc in range(4):
                        c = 4 * g + cc
                        csl = slice(128 * c, 128 * c + 128)
                        osl = slice(128 * cc, 128 * cc + 128)
                        for h in range(2):
                            hsl = slice(64 * h, 64 * h + 64)
                            st_ps = stp.tile([128, 128], F32, tag="st")
                            nc.tensor.matmul(
                                st_ps[:, :], kT[hsl, csl], qT[hsl, csl],
                                start=True, stop=True,
                            )
                            stm = work.tile([128, 128], BF16, tag="stm")
                            nc.vector.tensor_mul(stm, st_ps, mask)
                            # denominator row (head h at partition 64*h)
                            nc.tensor.matmul(
                                den_ps[64 * h : 64 * h + 1, osl],
                                ones_col[:, :], stm[:, :],
                                start=True, stop=(c == 0),
                                skip_group_check=True,
                            )
                            if c > 0:
                                nc.tensor.matmul(
                                    den_ps[64 * h : 64 * h + 1, osl],
                                    z_exc[hsl, c : c + 1], qT[hsl, csl],
                                    start=False, stop=True,
                                    skip_group_check=True,
                                )
                            # numerator^T: intra + inter
                            nc.tensor.matmul(
                                ot_ps[hsl, osl], v_nat[:, c, hsl], stm[:, :],
                                start=True, stop=(c == 0),
                                skip_group_check=True,
                            )
                            if c > 0:
                                nc.tensor.matmul(
                                    ot_ps[hsl, osl], state_sb[hsl, :], qT[hsl, csl],
                                    start=False, stop=True,
                                    skip_group_check=True,
                                )
                            # running state += K_c^T V_c  (head h at base 64h)
                            nc.tensor.matmul(
                                state_ps[hsl, :], k_nat[:, c, hsl], v_nat[:, c, hsl],
                                start=(c == 0), stop=(c == NCH - 1),
                                skip_group_check=True,
                            )
                        nsb = work.tile([128, 64], BF16, tag="stsb")
                        nc.vector.tensor_copy(nsb, state_ps[:, :])
                        state_sb = nsb
                    # divide by denominator: reciprocal rows -> broadcast -> mul
                    gsl = slice(512 * g, 512 * g + 512)
                    da = work.tile([1, 512], BF16, tag="da")
                    db = work.tile([1, 512], BF16, tag="db")
                    nc.scalar.activation(
                        out=da, in_=den_ps[0:1, :], func=AF.Copy,
                        bias=EPS_DEN, scale=1.0,
                    )
                    nc.scalar.activation(
                        out=db, in_=den_ps[64:65, :], func=AF.Copy,
                        bias=EPS_DEN, scale=1.0,
                    )
                    bc_ps = bcp2.tile([128, 512], F32, tag="bc2")
                    nc.tensor.matmul(bc_ps[0:64, :], ones_row64[:, :], da[:, :],
                                     start=True, stop=True, skip_group_check=True)
                    nc.tensor.matmul(bc_ps[64:128, :], ones_row64[:, :], db[:, :],
                                     start=True, stop=True, skip_group_check=True)
                    bcs = work.tile([128, 512], F32, tag="bcs")
                    nc.scalar.activation(out=bcs, in_=bc_ps, func=AF.Copy,
                                         bias=0.0, scale=1.0)
                    rec = work.tile([128, 512], F32, tag="rec")
                    nc.vector.reciprocal(rec, bcs)
                    nc.vector.tensor_mul(oT[:, gsl], ot_ps, rec)
                    # projection for this group's 4 chunks
                    for cc in range(4):
                        c = 4 * g + cc
                        for oh in range(2):
                            pj_ps = pjp.tile([128, 512], F32, tag="pj")
                            nc.tensor.matmul(
                                pj_ps[:, :],
                                oT[:, 128 * c : 128 * c + 128],
                                pws[:, 512 * oh : 512 * oh + 512],
                                start=True, stop=True,
                            )
                            yt = ypool.tile([128, 512], F32, tag="y")
                            nc.vector.tensor_copy(yt, pj_ps)
                            nc.sync.dma_start(
                                out=y_d[128 * c : 128 * c + 128,
                                        512 * oh : 512 * oh + 512],
                                in_=yt,
                            )
    nc.finalize()
    return nc


# ----------------------------------------------------------------------------
# host orchestration
# ----------------------------------------------------------------------------

_CACHE = {}


def _get_ncs():
    if "nc1" not in _CACHE:
        _CACHE["nc1"] = build_launch1()
        _CACHE["nc2"] = build_launch2()
    return _CACHE["nc1"], _CACHE["nc2"]


def kernel(x, ln_g, ln_b, qkv_w, qkv_b, gate_w, gate_b, proj_w, proj_b,
           _trace=False):
    _install_profile_hook()
    x = np.asarray(x)
    B = x.shape[0]
    xf = np.ascontiguousarray(x.reshape(T, D)).astype(np.float32)
    qkv_w = np.asarray(qkv_w, np.float32)
    qkv_b = np.asarray(qkv_b, np.float32)
    gate_w = np.asarray(gate_w, np.float32)
    gate_b = np.asarray(gate_b, np.float32)
    proj_w = np.asarray(proj_w, np.float32)
    proj_b = np.asarray(proj_b, np.float32)
    ln_g = np.asarray(ln_g, np.float32)
    ln_b = np.asarray(ln_b, np.float32)

    bf = ml_dtypes.bfloat16
    xT_bf = np.ascontiguousarray(xf.T).astype(bf)
    x_bf = xf.astype(bf)
    mask = np.triu(np.ones((C, C), np.float32))
    ident = np.eye(128, dtype=np.float32).astype(bf)

    nc1, nc2 = _get_ncs()

    in1, in2 = [], []
    per_core = []
    for c in range(NCORES):
        qs = slice(128 * c, 128 * c + 128)
        wq = qkv_w[qs, :] * ln_g[None, :]
        wk = qkv_w[1024 + 128 * c : 1024 + 128 * c + 128, :] * ln_g[None, :]
        wv = qkv_w[2048 + 128 * c : 2048 + 128 * c + 128, :] * ln_g[None, :]
        bq = qkv_w[qs, :] @ ln_b + qkv_b[qs]
        bk = qkv_w[1024 + 128 * c : 1024 + 128 * c + 128, :] @ ln_b \
            + qkv_b[1024 + 128 * c : 1024 + 128 * c + 128]
        bv = qkv_w[2048 + 128 * c : 2048 + 128 * c + 128, :] @ ln_b \
            + qkv_b[2048 + 128 * c : 2048 + 128 * c + 128]
        cgq = np.stack([wq.sum(1), bq]).astype(bf)          # [2, 128]
        cgk = np.stack([wk.sum(1), bk]).astype(bf)
        cgv = np.stack([wv.sum(1), bv]).astype(bf)
        wgt = np.ascontiguousarray(gate_w[qs, :].T).astype(bf)
        gbc = gate_b[qs].reshape(OPC, 1).astype(np.float32)
        pwt = np.ascontiguousarray(proj_w[:, qs].T).astype(bf)
        per_core.append((cgq, cgk, cgv,
                         np.ascontiguousarray(wq.T).astype(bf),
                         np.ascontiguousarray(wk.T).astype(bf),
                         np.ascontiguousarray(wv.T).astype(bf), pwt))
        in1.append({"xT": xT_bf, "wg": wgt, "gb": gbc})

    r1 = run_bass_kernel_spmd(nc1, in1, core_ids=list(range(NCORES)),
                              trace=_trace)
    gmean = np.sum([r1.results[c]["gcs"][0] for c in range(NCORES)],
                   axis=0).astype(np.float64) / D
    grec = (1.0 / (gmean + EPS_GATE)).astype(np.float32)      # [T]
    grec_nat = np.ascontiguousarray(grec.reshape(NCH, 128).T)  # [128, 16]

    for c in range(NCORES):
        cgq, cgk, cgv, wqt, wkt, wvt, pwt = per_core[c]
        in2.append({
            "xT": xT_bf, "xn": x_bf, "gsig": r1.results[c]["gsig"],
            "grec": grec_nat, "wq": wqt, "wk": wkt, "wv": wvt,
            "cgq": cgq, "cgk": cgk, "cgv": cgv, "pw": pwt,
            "mask": mask, "ident": ident,
        })
    r2 = run_bass_kernel_spmd(nc2, in2, core_ids=list(range(NCORES)),
                              trace=_trace)
    y = np.sum([r2.results[c]["y"] for c in range(NCORES)], axis=0)
    y = y + proj_b[None, :]
    out = y.reshape(B, T, D).astype(np.float32)
    if _trace:
        out = (out, r1.exec_time_ns, r2.exec_time_ns)
    return out
